# revision 1
# baseline (speedup 1.0000x reference)
"""Trainium2 Bass kernel for nn_MultiHeadAttention_78237124264578.

Reference computation (NO softmax — attention is purely bilinear):
    q = (x @ Wq.T + bq).reshape(8, 2, 2048, 64)   # FLAT reshape
    att = einsum('hbid,hbjd->hbij', q, k) * 64**-0.5
    out = einsum('hbij,hbjd->hbid', att, v)
    return out.transpose(1,2,3,0).reshape(2, 2048, 512)

Key identities exploited:
  1. (q kT) v == q (kT v): the 2048x2048 attention matrix collapses to a
     64x64 Gram matrix S = K^T V per (head, block b2).
  2. The head reshape is flat: head h / block b2 of Q/K/V is rows
     [512h + 256 b2, 512h + 256(b2+1)) of the [4096, 512] projection
     output, reinterpreted [256,512]->[2048,64].  So core i only needs
     x rows [512i, 512(i+1)) plus the full (512x512) weights.
  3. O_chunk[256r, 512f] = Yq_chunk @ (I8 (x) S): per 128-col chunk c the
     transposed output OT[128, 512r] = blockdiag(S, S)^T @ YqT_chunk, so
     one 128-partition matmul per (b2, chunk) computes O.

Sharding: head i -> core i.  All inputs shipped bf16 (halves the DMA,
matmul rate identical to fp32r, and small-free-size matmuls avoid the
fp32r 4x penalty).  The 0.125 attention scale is folded into Wq/bq on
the host.  Inputs are packed into ONE dram tensor in consumption order
so pipelined DMAs feed the PE without stalls (bias rows ride as tiny
DMAs between the xt/wk and wv blocks).  Yv c2/c3 run early so v3's bias
(the S1 gate) lands ~12.7us; the last Q chunk is computed/biased in
128-column quarter tiles (separate tiles — same-tile disjoint accesses
serialize in the dependency tracker); O pairs for c0/c1 run (and store)
before the q3 quarters so the final copy->store chain stays short.
PSUM bank recycling order is chosen so no matmul waits on a slot.
"""

import functools

import numpy as np

NCORES = 8
NIN = 512          # input features = contraction dim
NF = 512           # projection output features
R = 512            # rows per core (one head)
KC = NIN // 128    # contraction chunks
FC = NF // 128     # feature/row chunks
DIM = 64
SCALE = DIM ** -0.5

# PE warm-up matmuls issued before the real work (keeps the PE p-state
# ramping while the first input DMAs are in flight).
N_WARMUP = 5


@functools.lru_cache(maxsize=1)
def _build():
    from concourse import bacc
    import concourse.mybir as mybir
    import concourse.tile as tile

    f32 = mybir.dt.float32
    bf16 = mybir.dt.bfloat16

    nc = bacc.Bacc(None, target_bir_lowering=False)

    # packed operands: 16 slots of [128, 512] bf16, consumption order
    inp_d = nc.dram_tensor("inp", [128, 16, 512], bf16, kind="ExternalInput")
    brow_d = nc.dram_tensor("brow", [1, 2 * NF], f32, kind="ExternalInput")  # bk|bv
    bqc_d = nc.dram_tensor("bqc", [128, FC], f32, kind="ExternalInput")  # 0.125*bq
    ot_d = nc.dram_tensor("ot", [NF, R], bf16, kind="ExternalOutput")

    XT = [0, 2, 4, 6]   # xt k-chunk slots
    WK = [1, 3, 5, 7]
    WV = [8, 9, 10, 11]
    WQ = [12, 13, 14, 15]

    with tile.TileContext(nc) as tc:
        with (
            tc.tile_pool(name="sb", bufs=1) as sb,
            tc.tile_pool(name="pa", bufs=4, space="PSUM") as pa,
            tc.tile_pool(name="pb", bufs=4, space="PSUM") as pb,
        ):
            # ---- PE warm-up (Pool memset is ready ~600ns in) ---------------
            wu = sb.tile([1, 128], f32, tag="wu", name="wu")
            nc.gpsimd.memset(wu[:], 0.0)
            for i in range(N_WARMUP):
                psw = pb.tile([1, 128], f32, tag="B", name=f"psw{i}")
                nc.tensor.matmul(psw[:], wu[0:1, 0:1], wu[:])

            # ---- input DMAs on SP/HWDGE in consumption order; tiny bias
            # rows ride as DMAs #3/#4 (their transfers are ~23/56 ns) --------
            ops = sb.tile([128, 16, 512], bf16, tag="ops", name="ops")
            brow = sb.tile([1, 2 * NF], f32, tag="brow")
            bqc = sb.tile([128, FC], f32, tag="bqc")
            for t in range(4):
                nc.sync.dma_start(
                    ops[:, 2 * t:2 * t + 2, :], inp_d[:, 2 * t:2 * t + 2, :])
            nc.sync.dma_start(brow[:], brow_d[:, :])
            nc.sync.dma_start(bqc[:], bqc_d[:, :])
            for t in range(4, 8):
                nc.sync.dma_start(
                    ops[:, 2 * t:2 * t + 2, :], inp_d[:, 2 * t:2 * t + 2, :])

            bkb = sb.tile([128, NF], f32, tag="bkb")
            bvb = sb.tile([128, NF], f32, tag="bvb")
            s2b = [sb.tile([128, 128], bf16, tag=f"s2b{b}", name=f"s2b{b}")
                   for b in range(2)]
            nc.gpsimd.memset(s2b[0][:], 0.0)
            nc.gpsimd.memset(s2b[1][:], 0.0)
            nc.gpsimd.partition_broadcast(bkb[:], brow[0:1, 0:NF])
            nc.gpsimd.partition_broadcast(bvb[:], brow[0:1, NF:2 * NF])

            k_sb = [sb.tile([128, NF], bf16, tag=f"k{c}", name=f"k{c}") for c in range(FC)]
            v_sb = [sb.tile([128, NF], bf16, tag=f"v{c}", name=f"v{c}") for c in range(FC)]
            # q chunks 0..2 full; chunk 3 as four separate quarter tiles
            q_sb = [sb.tile([128, R], bf16, tag=f"q{c}", name=f"q{c}") for c in range(3)]
            q3_sb = [sb.tile([128, 128], bf16, tag=f"q3_{j}", name=f"q3_{j}")
                     for j in range(4)]

            def slot(s):
                return ops[:, s, :]

            def slotc(s, c):
                return ops[:, s, 128 * c:128 * (c + 1)]

            # ---- Yk: psk[c][r,f], k-outer to match DMA arrival -------------
            psk = [pa.tile([128, NF], f32, tag="A", name=f"psk{c}") for c in range(FC)]
            for k in range(KC):
                for c in range(FC):
                    nc.tensor.matmul(
                        psk[c][:], slotc(XT[k], c), slot(WK[k]),
                        start=(k == 0), stop=(k == KC - 1),
                    )
            # K bias adds: PSUM tensor+tensor is DVE-only; c0/c1 first (S0)
            nc.vector.tensor_add(k_sb[0][:], psk[0][:], bkb[:])
            nc.vector.tensor_add(k_sb[1][:], psk[1][:], bkb[:])

            psv = [pb.tile([128, NF], f32, tag="B", name=f"psv{c}") for c in range(FC)]

            def yv(c):
                for k in range(KC):
                    nc.tensor.matmul(
                        psv[c][:], slotc(XT[k], c), slot(WV[k]),
                        start=(k == 0), stop=(k == KC - 1),
                    )

            def vbias(c):
                nc.vector.tensor_add(v_sb[c][:], psv[c][:], bvb[:])

            psq = [pa.tile([128, R], f32, tag="A", name=f"psq{c}") for c in range(3)]
            q3_ps = [pa.tile([128, 128], f32, tag="A", name=f"q3ps{j}")
                     for j in range(2)]

            def yq(c):
                for k in range(KC):
                    nc.tensor.matmul(
                        psq[c][:], slotc(WQ[k], c), slot(XT[k]),
                        start=(k == 0), stop=(k == KC - 1),
                    )

            def yq3(j):
                sl = slice(128 * j, 128 * (j + 1))
                for k in range(KC):
                    nc.tensor.matmul(
                        q3_ps[j][:], slotc(WQ[k], 3), slot(XT[k])[:, sl],
                        start=(k == 0), stop=(k == KC - 1),
                    )

            def qbias_act(c):
                nc.scalar.activation(
                    q_sb[c][:], psq[c][:],
                    mybir.ActivationFunctionType.Identity,
                    bias=bqc[:, c:c + 1], scale=1.0,
                )

            def s_mm(ps_s, b2):
                idx = 0
                for rc in (2 * b2, 2 * b2 + 1):
                    for fh in range(8):
                        nc.tensor.matmul(
                            ps_s[:],
                            k_sb[rc][:, 64 * fh:64 * (fh + 1)],
                            v_sb[rc][:, 64 * fh:64 * (fh + 1)],
                            start=(idx == 0), stop=(idx == 15),
                        )
                        idx += 1

            # ---- pipeline: Yv c2/c3 pulled early so v3-add (the S1 gate)
            # lands ~12.7; S1 right after Yq c1; Yq c2 + q3 quarters fill
            # PE while s2b1/q biases land; O matmuls bunched at the end ----
            yv(0)
            yv(1)
            vbias(0)
            vbias(1)
            yq(0)
            qbias_act(0)
            yv(2)
            vbias(2)
            ps_s0 = pb.tile([64, 64], f32, tag="B", name="ps_s0")
            s_mm(ps_s0, 0)
            nc.vector.tensor_add(k_sb[2][:], psk[2][:], bkb[:])
            nc.vector.tensor_add(k_sb[3][:], psk[3][:], bkb[:])
            nc.scalar.copy(s2b[0][0:64, 0:64], ps_s0[:])
            nc.scalar.copy(s2b[0][64:128, 64:128], ps_s0[:])
            yv(3)
            vbias(3)
            yq(1)
            qbias_act(1)
            ps_s1 = pb.tile([64, 64], f32, tag="B", name="ps_s1")
            q3_ps.append(pb.tile([128, 128], f32, tag="B", name="q3ps2"))
            q3_ps.append(pb.tile([128, 128], f32, tag="B", name="q3ps3"))
            s_mm(ps_s1, 1)
            nc.vector.tensor_copy(s2b[1][0:64, 0:64], ps_s1[:])
            nc.vector.tensor_copy(s2b[1][64:128, 64:128], ps_s1[:])
            yq(2)
            qbias_act(2)

            # output chunks
            ps_oc = [None] * 3
            ps_oc[0] = pb.tile([128, R], f32, tag="B", name="ps_oc0")
            ps_oc[1] = pb.tile([128, R], f32, tag="B", name="ps_oc1")
            ps_oc[2] = pa.tile([128, R], f32, tag="A", name="ps_oc2")
            ps_o3 = [pa.tile([128, 256], f32, tag="A", name=f"ps_o3{h}")
                     for h in range(2)]
            oc_sb = [sb.tile([128, R], bf16, tag=f"oc{c}", name=f"oc{c}")
                     for c in range(3)]
            oc3 = sb.tile([128, R], bf16, tag="oc3", name="oc3")

            def o_mm(c, b2):
                rsl = slice(256 * b2, 256 * (b2 + 1))
                nc.tensor.matmul(ps_oc[c][:, rsl], s2b[b2][:], q_sb[c][:, rsl])

            o_mm(0, 0)
            o_mm(0, 1)
            nc.vector.tensor_copy(oc_sb[0][:], ps_oc[0][:])
            nc.sync.dma_start(ot_d[0:128, :], oc_sb[0][:])
            o_mm(1, 0)
            o_mm(1, 1)
            nc.scalar.copy(oc_sb[1][:], ps_oc[1][:])
            nc.gpsimd.dma_start(ot_d[128:256, :], oc_sb[1][:])
            yq3(0)
            nc.vector.tensor_scalar_add(q3_sb[0][:], q3_ps[0][:], bqc[:, 3:4])
            yq3(1)
            nc.scalar.activation(
                q3_sb[1][:], q3_ps[1][:],
                mybir.ActivationFunctionType.Identity,
                bias=bqc[:, 3:4], scale=1.0,
            )
            yq3(2)
            nc.vector.tensor_scalar_add(q3_sb[2][:], q3_ps[2][:], bqc[:, 3:4])
            yq3(3)
            nc.scalar.activation(
                q3_sb[3][:], q3_ps[3][:],
                mybir.ActivationFunctionType.Identity,
                bias=bqc[:, 3:4], scale=1.0,
            )
            o_mm(2, 0)
            o_mm(2, 1)
            for j in range(4):
                h, col = j // 2, (j % 2) * 128
                nc.tensor.matmul(
                    ps_o3[h][:, col:col + 128], s2b[h][:], q3_sb[j][:])

            nc.vector.tensor_copy(oc3[:, 0:256], ps_o3[0][:])
            nc.scalar.copy(oc3[:, 256:512], ps_o3[1][:])
            nc.sync.dma_start(ot_d[384:512, :], oc3[:])
            nc.vector.tensor_copy(oc_sb[2][:], ps_oc[2][:])
            nc.sync.dma_start(ot_d[256:384, :], oc_sb[2][:])

    nc.compile()
    return nc


def kernel(x, Wq, bq, Wk, bk, Wv, bv):
    import ml_dtypes
    from concourse.bass_utils import run_bass_kernel_spmd

    bf16 = ml_dtypes.bfloat16
    x = np.asarray(x, dtype=np.float32)
    Wq = np.asarray(Wq, dtype=np.float32)
    Wk = np.asarray(Wk, dtype=np.float32)
    Wv = np.asarray(Wv, dtype=np.float32)
    bq = np.asarray(bq, dtype=np.float32)
    bk = np.asarray(bk, dtype=np.float32)
    bv = np.asarray(bv, dtype=np.float32)

    B, N, nin = x.shape
    x_flat = x.reshape(B * N, nin)                       # [4096, 512]

    wkt = Wk.T.astype(bf16)                              # [k, f]
    wvt = Wv.T.astype(bf16)
    wqt = (SCALE * Wq).T.astype(bf16)
    brow = np.ascontiguousarray(
        np.concatenate([bk, bv]).reshape(1, 2 * NF))
    bqc = np.ascontiguousarray((SCALE * bq).reshape(FC, 128).T)  # [p, c]

    def chunks(t):
        return [t[128 * j:128 * (j + 1)] for j in range(4)]

    wk_c, wv_c, wq_c = chunks(wkt), chunks(wvt), chunks(wqt)

    in_maps = []
    for i in range(NCORES):
        xt_i = x_flat[R * i:R * (i + 1)].T.astype(bf16)  # [k, r]
        xt_c = chunks(xt_i)
        slots = [xt_c[0], wk_c[0], xt_c[1], wk_c[1],
                 xt_c[2], wk_c[2], xt_c[3], wk_c[3],
                 *wv_c, *wq_c]
        inp = np.ascontiguousarray(np.stack(slots, axis=1))  # [128, 16, 512]
        in_maps.append({"inp": inp, "brow": brow, "bqc": bqc})

    nc = _build()
    res = run_bass_kernel_spmd(nc, in_maps, core_ids=list(range(NCORES)))

    # ot[i][f_hi*64+d, b2*256+rr] = out[h=i, b2, n2=rr*8+f_hi, d]
    ot = np.stack([np.asarray(res.results[i]["ot"], dtype=np.float32)
                   for i in range(NCORES)])                       # [h, f', r]
    ot = ot.reshape(NCORES, 8, DIM, 2, 256)                       # [h, fh, d, b2, rr]
    z = ot.transpose(3, 4, 1, 2, 0).reshape(B, N, 8 * DIM)        # [b2, n2, d*8+h]
    return np.ascontiguousarray(z)



# revision 2
# speedup vs baseline: 1.0204x; 1.0204x over previous
"""Trainium2 Bass kernel for nn_MultiHeadAttention_78237124264578.

Reference computation (NO softmax; attention is purely bilinear):
    q = (x @ Wq.T + bq).reshape(8, 2, 2048, 64)   # FLAT reshape
    att = einsum('hbid,hbjd->hbij', q, k) * 64**-0.5
    out = einsum('hbij,hbjd->hbid', att, v)
    return out.transpose(1,2,3,0).reshape(2, 2048, 512)

Identities exploited (same as the bf16 baseline):
  1. (q kT) v == q (kT v): the attention matrix collapses to a 64x64
     Gram matrix S = K^T V per (head, 256-row block b2).
  2. The head reshape is flat: head h of Q/K/V is rows [512h, 512h+512)
     of the [4096, 512] projection output, so core i only needs x rows
     [512i, 512(i+1)) plus the full 512x512 weight matrices.
  3. O^T[f', r] per 128-row chunk is one matmul with the block-diagonal
     [S; S] as the stationary operand.

Speed trick on top: fp8e4 DoubleRow matmuls (2 contraction tiles per
instruction at 0.5 cycles/row -> 4x bf16 throughput in the cost model).
Full fp8 is too lossy (6.6% rel err), so every projection is computed
as a 3-term compensated product

    Y*256 = xh@Wh + xh@Wl + xl@Wh,   xh=fp8(x),     xl=fp8(x-xh)
                                     Wh=fp8(256 W), Wl=fp8(256 W - Wh)

which lands at ~0.4% overall rel err (bf16-comparable) at 0.75x the
bf16 PE cycle count (18432 -> plus small S/O stages in bf16).
Weights are scaled by 256 so fp8 normals cover them; the scale is
unwound on the host (output / 2^24) since S and O inherit 256^2 and
256^3 factors.  K/V biases (which vary along the free dim, so neither
ACT-bias nor tensor_scalar ops can apply them) ride INSIDE the psum
accumulation as a rank-1 DoubleRow term outer(ones, bias_hi+bias_lo),
making every PSUM->SBUF drain a plain copy that can be split across
the ACT and DVE engines (Pool cannot read PSUM).  Q bias varies along
partitions and uses ACT activation-bias / DVE tensor_scalar_add.

Input is packed into one [128, 32, 512] fp8 dram tensor in consumption
order, fetched with eight 4-slot DMAs on the SP/HWDGE queue; the two
small bias tensors ride the Pool/SWDGE queue so they never contend for
HWDGE.  Output ships as four [128, 512] bf16 DMAs as O chunks drain.
"""

import functools

import numpy as np

NCORES = 8
NIN = 512
NF = 512
R = 512
DIM = 64
SCALE = DIM ** -0.5
WS = 256.0           # weight scale so fp8e4 sees normal-range values
OUT_DESCALE = 1.0 / (WS * WS * WS)

N_WARMUP = 5

# pair-base slots in the packed input (pair i = slots [b, b+1])
XH = (0, 4)      # xh chunks (c0,c1) and (c2,c3)
WKH = (2, 6)
WKL = (8, 10)
XL = (12, 14)
WVH = (16, 18)
WVL = (20, 22)
WQH = (24, 26)
WQL = (28, 30)


@functools.lru_cache(maxsize=1)
def _build():
    from concourse import bacc
    import concourse.mybir as mybir
    import concourse.tile as tile

    f32 = mybir.dt.float32
    bf16 = mybir.dt.bfloat16
    f8 = mybir.dt.float8e4
    DR = mybir.MatmulPerfMode.DoubleRow
    IDT = mybir.ActivationFunctionType.Identity

    nc = bacc.Bacc(None, target_bir_lowering=False)

    inp_d = nc.dram_tensor("inp", [128, 32, 512], f8, kind="ExternalInput")
    # bias8 rows: 0 ones, 1 ones, 2 bkh, 3 bkl, 4 bvh, 5 bvl  (fp8, x256)
    bias8_d = nc.dram_tensor("bias8", [1, 6, 512], f8, kind="ExternalInput")
    bqc_d = nc.dram_tensor("bqc", [128, 4], f32, kind="ExternalInput")  # 256*SCALE*bq
    ot_d = nc.dram_tensor("ot", [NF, R], bf16, kind="ExternalOutput")

    with tile.TileContext(nc) as tc:
        with (
            tc.tile_pool(name="sb", bufs=1) as sb,
            tc.tile_pool(name="pa", bufs=4, space="PSUM") as pa,
            tc.tile_pool(name="pb", bufs=4, space="PSUM") as pb,
        ):
            # ---- PE warm-up: start the p-state ramp early -----------------
            wu = sb.tile([1, 128], f32, tag="wu", name="wu")
            nc.gpsimd.memset(wu[:], 0.0)
            for i in range(N_WARMUP):
                psw = pb.tile([1, 128], f32, tag="B", name=f"psw{i}")
                nc.tensor.matmul(psw[:], wu[0:1, 0:1], wu[:])

            # ---- DMAs ------------------------------------------------------
            ops = sb.tile([128, 32, 512], f8, tag="ops", name="ops")
            bias8 = sb.tile([1, 6, 512], f8, tag="bias8")
            bqc = sb.tile([128, 4], f32, tag="bqc")
            nc.gpsimd.dma_start(bias8[:], bias8_d[:, :, :])
            nc.gpsimd.dma_start(bqc[:], bqc_d[:, :])
            for t in range(8):
                nc.sync.dma_start(
                    ops[:, 4 * t:4 * t + 4, :], inp_d[:, 4 * t:4 * t + 4, :])

            s2b = [sb.tile([128, 128], bf16, tag=f"s2b{b}", name=f"s2b{b}")
                   for b in range(2)]
            nc.gpsimd.memset(s2b[0][:], 0.0)
            nc.gpsimd.memset(s2b[1][:], 0.0)

            k_sb = [sb.tile([128, NF], bf16, tag=f"k{c}", name=f"k{c}") for c in range(4)]
            v_sb = [sb.tile([128, NF], bf16, tag=f"v{c}", name=f"v{c}") for c in range(4)]
            q_sb = [sb.tile([128, R], bf16, tag=f"q{c}", name=f"q{c}") for c in range(4)]
            oc_sb = [sb.tile([128, R], bf16, tag=f"oc{c}", name=f"oc{c}")
                     for c in range(4)]

            def pair(base, cols):
                return ops[:, base:base + 2, cols]

            # K/V: out[row-chunk rc, f]; stationary x pair, moving w pair
            def dr_kv(ps, rc, fh, xs, ws, start=False, stop=False):
                nc.tensor.matmul(
                    ps[:, 256 * fh:256 * fh + 256],
                    pair(xs, slice(128 * rc, 128 * rc + 128)),
                    pair(ws, slice(256 * fh, 256 * fh + 256)),
                    start=start, stop=stop, perf_mode=DR,
                )

            # Q: out[f-chunk c, r]; stationary w pair, moving x pair
            def dr_q(ps, c, rh, ws, xs, start=False, stop=False):
                nc.tensor.matmul(
                    ps[:, 256 * rh:256 * rh + 256],
                    pair(ws, slice(128 * c, 128 * c + 128)),
                    pair(xs, slice(256 * rh, 256 * rh + 256)),
                    start=start, stop=stop, perf_mode=DR,
                )

            # rank-1 bias term: outer(ones, bias_hi) + outer(ones, bias_lo)
            def dr_bias(ps, fh, brow):
                nc.tensor.matmul(
                    ps[:, 256 * fh:256 * fh + 256],
                    bias8[0:1, 0:2, 0:128],
                    bias8[0:1, brow:brow + 2, 256 * fh:256 * fh + 256],
                    start=False, stop=False, perf_mode=DR,
                )

            # ---- K projection ---------------------------------------------
            psk = [pa.tile([128, NF], f32, tag="A", name=f"psk{c}") for c in range(4)]
            for rc in range(4):                      # t1 p0 [needs D1]
                for fh in range(2):
                    dr_kv(psk[rc], rc, fh, XH[0], WKH[0], start=(fh == 0))
            for rc in range(4):                      # bias rows [tiny DMA]
                for fh in range(2):
                    dr_bias(psk[rc], fh, 2)
            for rc in range(4):                      # t1 p1 [D2]
                for fh in range(2):
                    dr_kv(psk[rc], rc, fh, XH[1], WKH[1])
            for p in range(2):                       # t2 [D3]
                for rc in range(4):
                    for fh in range(2):
                        dr_kv(psk[rc], rc, fh, XH[p], WKL[p])
            for rc in range(4):                      # t3 bank-major [D4]
                for p in range(2):
                    for fh in range(2):
                        dr_kv(psk[rc], rc, fh, XL[p], WKH[p],
                              stop=(p == 1 and fh == 1))
                if rc % 2 == 0:
                    nc.scalar.copy(k_sb[rc][:], psk[rc][:])
                else:
                    nc.vector.tensor_copy(k_sb[rc][:], psk[rc][:])

            # ---- V projection ---------------------------------------------
            psv = [pb.tile([128, NF], f32, tag="B", name=f"psv{c}") for c in range(4)]
            for rc in range(4):                      # t1 p0/p1 [D5]
                for fh in range(2):
                    dr_kv(psv[rc], rc, fh, XH[0], WVH[0], start=(fh == 0))
            for rc in range(4):
                for fh in range(2):
                    dr_bias(psv[rc], fh, 4)
            for rc in range(4):
                for fh in range(2):
                    dr_kv(psv[rc], rc, fh, XH[1], WVH[1])
            for p in range(2):                       # t3 (xl already in)
                for rc in range(4):
                    for fh in range(2):
                        dr_kv(psv[rc], rc, fh, XL[p], WVH[p])
            for rc in range(4):                      # t2 bank-major [D6]
                for p in range(2):
                    for fh in range(2):
                        dr_kv(psv[rc], rc, fh, XH[p], WVL[p],
                              stop=(p == 1 and fh == 1))
                if rc % 2 == 0:
                    nc.scalar.copy(v_sb[rc][:], psv[rc][:])
                else:
                    nc.vector.tensor_copy(v_sb[rc][:], psv[rc][:])

            # ---- Q projection t1+t3 [D7] ----------------------------------
            psq = [pa.tile([128, R], f32, tag="A", name=f"psq{c}") for c in range(4)]
            for c in range(4):
                for rh in range(2):
                    dr_q(psq[c], c, rh, WQH[0], XH[0], start=(rh == 0))
            for c in range(4):
                for rh in range(2):
                    dr_q(psq[c], c, rh, WQH[1], XH[1])
            for p in range(2):
                for c in range(4):
                    for rh in range(2):
                        dr_q(psq[c], c, rh, WQH[p], XL[p])

            # ---- S = K^T V per b2 -----------------------------------------
            def s_mm(ps_s, b2):
                idx = 0
                for rc in (2 * b2, 2 * b2 + 1):
                    for fh8 in range(8):
                        nc.tensor.matmul(
                            ps_s[:],
                            k_sb[rc][:, 64 * fh8:64 * fh8 + 64],
                            v_sb[rc][:, 64 * fh8:64 * fh8 + 64],
                            start=(idx == 0), stop=(idx == 15),
                        )
                        idx += 1

            ps_s0 = pb.tile([64, 64], f32, tag="B", name="ps_s0")
            s_mm(ps_s0, 0)
            nc.scalar.copy(s2b[0][0:64, 0:64], ps_s0[:])
            nc.scalar.copy(s2b[0][64:128, 64:128], ps_s0[:])
            ps_s1 = pb.tile([64, 64], f32, tag="B", name="ps_s1")
            s_mm(ps_s1, 1)
            nc.vector.tensor_copy(s2b[1][0:64, 0:64], ps_s1[:])
            nc.vector.tensor_copy(s2b[1][64:128, 64:128], ps_s1[:])

            # ---- Q t2 bank-major [D8] + bias + O --------------------------
            for c in range(4):
                for p in range(2):
                    for rh in range(2):
                        dr_q(psq[c], c, rh, WQL[p], XH[p],
                             stop=(p == 1 and rh == 1))
                if c % 2 == 0:
                    nc.scalar.activation(
                        q_sb[c][:], psq[c][:], IDT,
                        bias=bqc[:, c:c + 1], scale=1.0,
                    )
                else:
                    nc.vector.tensor_scalar_add(
                        q_sb[c][:], psq[c][:], bqc[:, c:c + 1])

            ps_oc = [pb.tile([128, R], f32, tag="B", name=f"ps_oc{c}")
                     for c in range(4)]

            def o_mm(c, b2):
                rsl = slice(256 * b2, 256 * (b2 + 1))
                nc.tensor.matmul(ps_oc[c][:, rsl], s2b[b2][:], q_sb[c][:, rsl])

            for c in range(4):
                o_mm(c, 0)
                o_mm(c, 1)
                if c % 2 == 0:
                    nc.scalar.copy(oc_sb[c][:], ps_oc[c][:])
                else:
                    nc.vector.tensor_copy(oc_sb[c][:], ps_oc[c][:])
                nc.sync.dma_start(ot_d[128 * c:128 * c + 128, :], oc_sb[c][:])

    nc.compile()
    return nc


def kernel(x, Wq, bq, Wk, bk, Wv, bv):
    import ml_dtypes
    from concourse.bass_utils import run_bass_kernel_spmd

    f8 = ml_dtypes.float8_e4m3

    x = np.asarray(x, dtype=np.float32)
    Wq = np.asarray(Wq, dtype=np.float32)
    Wk = np.asarray(Wk, dtype=np.float32)
    Wv = np.asarray(Wv, dtype=np.float32)
    bq = np.asarray(bq, dtype=np.float32)
    bk = np.asarray(bk, dtype=np.float32)
    bv = np.asarray(bv, dtype=np.float32)

    B, N, nin = x.shape
    x_flat = x.reshape(B * N, nin)

    def split8(a):
        hi = np.asarray(a, f8)
        lo = np.asarray(a - hi.astype(np.float32), f8)
        return hi, lo

    def chunks(t):
        return [t[128 * j:128 * (j + 1)] for j in range(4)]

    wkh, wkl = split8(WS * Wk.T)
    wvh, wvl = split8(WS * Wv.T)
    wqh, wql = split8(WS * SCALE * Wq.T)
    wkh_c, wkl_c = chunks(wkh), chunks(wkl)
    wvh_c, wvl_c = chunks(wvh), chunks(wvl)
    wqh_c, wql_c = chunks(wqh), chunks(wql)

    bkh, bkl = split8(WS * bk)
    bvh, bvl = split8(WS * bv)
    ones = np.ones(512, f8)
    bias8 = np.ascontiguousarray(
        np.stack([ones, ones, bkh, bkl, bvh, bvl])[None, :, :])
    bqc = np.ascontiguousarray(
        (WS * SCALE * bq).reshape(4, 128).T)              # [p, c] f32

    in_maps = []
    for i in range(NCORES):
        xt = x_flat[R * i:R * (i + 1)].T                  # [in, r]
        xh, xl = split8(xt)
        xh_c, xl_c = chunks(xh), chunks(xl)
        slots = [
            xh_c[0], xh_c[1], wkh_c[0], wkh_c[1],
            xh_c[2], xh_c[3], wkh_c[2], wkh_c[3],
            wkl_c[0], wkl_c[1], wkl_c[2], wkl_c[3],
            xl_c[0], xl_c[1], xl_c[2], xl_c[3],
            wvh_c[0], wvh_c[1], wvh_c[2], wvh_c[3],
            wvl_c[0], wvl_c[1], wvl_c[2], wvl_c[3],
            wqh_c[0], wqh_c[1], wqh_c[2], wqh_c[3],
            wql_c[0], wql_c[1], wql_c[2], wql_c[3],
        ]
        inp = np.ascontiguousarray(np.stack(slots, axis=1))  # [128, 32, 512]
        in_maps.append({"inp": inp, "bias8": bias8, "bqc": bqc})

    nc = _build()
    res = run_bass_kernel_spmd(nc, in_maps, core_ids=list(range(NCORES)))

    # ot[i][fh*64+d, b2*256+rr] = 2^24 * out[h=i, b2, n2=rr*8+fh, d]
    ot = np.stack([np.asarray(res.results[i]["ot"], dtype=np.float32)
                   for i in range(NCORES)])                       # [h, f', r]
    ot *= OUT_DESCALE
    ot = ot.reshape(NCORES, 8, DIM, 2, 256)                       # [h, fh, d, b2, rr]
    z = ot.transpose(3, 4, 1, 2, 0).reshape(B, N, 8 * DIM)        # [b2, n2, d*8+h]
    return np.ascontiguousarray(z)
